# revision 1
# baseline (speedup 1.0000x reference)
"""Llama GQA attention (B=2,S=1024,P=1024,E=2048,H=32,KV=8,HD=64) on 8 TRN2 cores.

Sharding: tensor-parallel on the KV-group axis — core c owns KV group c and its
4 query heads. x / cos / sin / mask replicated; Wq/Wk/Wv row-sharded; Wo
column-sharded (partial outputs summed on host); cache sharded on the KV axis.
"""
import os
import sys

for _p in ("/opt/trn_rl_repo",):
    if os.path.isdir(_p) and _p not in sys.path:
        sys.path.insert(0, _p)

import numpy as np
import ml_dtypes

import concourse.bass as bass
import concourse.tile as tile
from concourse import bacc, mybir
from concourse.bass_utils import run_bass_kernel_spmd

B, S, P, E, H, KV, HD = 2, 1024, 1024, 2048, 32, 8, 64
CTX = P + S            # 2048
G = H // KV            # 4 heads per core
T = B * S              # 2048 flattened tokens
N_CORES = 8
OC = G * HD            # 256 output cols per core (q / attn)
BF = mybir.dt.bfloat16
F32 = mybir.dt.float32
nbf = ml_dtypes.bfloat16

NCH = CTX // 128       # 16 key chunks of 128
NTQB = S // 512        # 2 query blocks of 512
NE = E // 128          # 16 embed chunks

_built = {}            # classification key -> compiled Bass module


def _classify(MT):
    """MT = exp(mask).T, shape [CTX, S]. Per (tqb, chunk): 'ones'|'zero'|'mixed'."""
    cls = {}
    for tqb in range(NTQB):
        for c in range(NCH):
            sub = MT[128 * c:128 * (c + 1), 512 * tqb:512 * (tqb + 1)]
            if np.all(sub == 1.0):
                cls[(tqb, c)] = "ones"
            elif np.all(sub == 0.0):
                cls[(tqb, c)] = "zero"
            else:
                cls[(tqb, c)] = "mixed"
    return cls


def _groups(cls, tqb):
    """Key-chunk groups of 4 for one tq block; a group is skipped only if all
    4 chunks are fully masked ('zero')."""
    out = []
    for g in range(NCH // 4):
        chunks = list(range(4 * g, 4 * g + 4))
        if all(cls[(tqb, c)] == "zero" for c in chunks):
            continue
        out.append(chunks)
    return out


def _build(cls, mixed_list):
    Exp = mybir.ActivationFunctionType.Exp
    midx = {tc: j for j, tc in enumerate(mixed_list)}
    nc = bacc.Bacc(None, target_bir_lowering=False, debug=False)

    xT = nc.dram_tensor("xT", [E, T], BF, kind="ExternalInput")
    wqT = nc.dram_tensor("wqT", [E, OC], BF, kind="ExternalInput")
    wkT = nc.dram_tensor("wkT", [E, HD], BF, kind="ExternalInput")
    wvT = nc.dram_tensor("wvT", [E, HD], BF, kind="ExternalInput")
    woT = nc.dram_tensor("woT", [OC, E], BF, kind="ExternalInput")
    cosR = nc.dram_tensor("cosR", [B, OC, S], BF, kind="ExternalInput")
    ssinR = nc.dram_tensor("ssinR", [B, OC, S], BF, kind="ExternalInput")
    cacheTk = nc.dram_tensor("cacheTk", [B, HD, P], BF, kind="ExternalInput")
    cacheV = nc.dram_tensor("cacheV", [B, P, HD], BF, kind="ExternalInput")
    if mixed_list:
        maskM = nc.dram_tensor("maskM", [len(mixed_list), 128, 512], BF,
                               kind="ExternalInput")
    out_part = nc.dram_tensor("out_part", [T, E], F32, kind="ExternalOutput")

    with tile.TileContext(nc) as tc:
        with (
            tc.tile_pool(name="persist", bufs=1) as pp,
            tc.tile_pool(name="work", bufs=3) as wp,
            tc.tile_pool(name="probs", bufs=3) as prp,
            tc.tile_pool(name="attn", bufs=4) as ap,
            tc.tile_pool(name="ostage", bufs=4) as op_,
            tc.tile_pool(name="ps_sc", bufs=1, space="PSUM") as ps_sc,
            tc.tile_pool(name="ps_av", bufs=2, space="PSUM") as ps_av,
            tc.tile_pool(name="ps_mm", bufs=2, space="PSUM") as ps_mm,
        ):
            # ---- persistent loads ----
            xt = []
            for i in range(NE):
                t_ = pp.tile([128, T], BF, tag=f"xt{i}")
                nc.sync.dma_start(t_[:], xT[128 * i:128 * (i + 1), :])
                xt.append(t_)
            wq = []
            for i in range(NE):
                t_ = pp.tile([128, OC], BF, tag=f"wq{i}")
                nc.sync.dma_start(t_[:], wqT[128 * i:128 * (i + 1), :])
                wq.append(t_)
            wk, wv = [], []
            for i in range(NE):
                t_ = pp.tile([128, HD], BF, tag=f"wk{i}")
                nc.sync.dma_start(t_[:], wkT[128 * i:128 * (i + 1), :])
                wk.append(t_)
                t_ = pp.tile([128, HD], BF, tag=f"wv{i}")
                nc.sync.dma_start(t_[:], wvT[128 * i:128 * (i + 1), :])
                wv.append(t_)
            wo = []
            for i in range(2):
                t_ = pp.tile([128, E], BF, tag=f"wo{i}")
                nc.sync.dma_start(t_[:], woT[128 * i:128 * (i + 1), :])
                wo.append(t_)
            cs, sn = [], []
            for b in range(B):
                cb, sb_ = [], []
                for hp in range(2):
                    t_ = pp.tile([128, S], BF, tag=f"cos{b}{hp}")
                    nc.sync.dma_start(t_[:], cosR[b, 128 * hp:128 * (hp + 1), :])
                    cb.append(t_)
                    t_ = pp.tile([128, S], BF, tag=f"sin{b}{hp}")
                    nc.sync.dma_start(t_[:], ssinR[b, 128 * hp:128 * (hp + 1), :])
                    sb_.append(t_)
                cs.append(cb)
                sn.append(sb_)
            keys, vals = [], []
            for b in range(B):
                kt = pp.tile([128, CTX], BF, tag=f"keys{b}")
                nc.sync.dma_start(kt[0:64, 0:P], cacheTk[b])
                nc.sync.dma_start(kt[64:128, 0:P], cacheTk[b])
                keys.append(kt)
                vt = pp.tile([128, NCH * 65], BF, tag=f"vals{b}")
                for k in range(P // 128):
                    nc.sync.dma_start(vt[:, 65 * k:65 * k + 64],
                                      cacheV[b, 128 * k:128 * (k + 1), :])
                for k in range(NCH):
                    nc.vector.memset(vt[:, 65 * k + 64:65 * k + 65], 1.0)
                vals.append(vt)
            mt = []
            for j in range(len(mixed_list)):
                t_ = pp.tile([128, 512], BF, tag=f"mask{j}")
                nc.sync.dma_start(t_[:], maskM[j])
                mt.append(t_)

            # ---- projections + RoPE ----
            qp = [[None, None] for _ in range(B)]
            for b in range(B):
                tok0 = b * S
                # k projection (transposed) + rope -> keys[b][:, P:]
                for tq2 in range(2):
                    sl = slice(tok0 + 512 * tq2, tok0 + 512 * (tq2 + 1))
                    ps = ps_mm.tile([128, 512], F32, tag="mm512")
                    for e in range(NE):
                        nc.tensor.matmul(ps[0:64, :], wk[e][:, 0:64], xt[e][:, sl],
                                         start=(e == 0), stop=(e == NE - 1))
                    kraw = wp.tile([64, 512], BF, tag="kraw")
                    nc.any.tensor_copy(kraw[:], ps[0:64, :])
                    ksw = wp.tile([64, 512], BF, tag="ksw")
                    nc.sync.dma_start(ksw[0:32, :], kraw[32:64, :])
                    nc.sync.dma_start(ksw[32:64, :], kraw[0:32, :])
                    t1 = wp.tile([64, 512], BF, tag="kt1")
                    nc.vector.tensor_mul(t1[:], kraw[:], cs[b][0][0:64, 512 * tq2:512 * (tq2 + 1)])
                    t2 = wp.tile([64, 512], BF, tag="kt2")
                    nc.vector.tensor_mul(t2[:], ksw[:], sn[b][0][0:64, 512 * tq2:512 * (tq2 + 1)])
                    ksl = slice(P + 512 * tq2, P + 512 * (tq2 + 1))
                    nc.vector.tensor_add(keys[b][0:64, ksl], t1[:], t2[:])
                    nc.sync.dma_start(keys[b][64:128, ksl], keys[b][0:64, ksl])
                # q projection (transposed, head-pair packed) + rope
                for hp in range(2):
                    qt = pp.tile([128, S], BF, tag=f"qp{b}{hp}")
                    qp[b][hp] = qt
                    for tq2 in range(2):
                        sl = slice(tok0 + 512 * tq2, tok0 + 512 * (tq2 + 1))
                        ps = ps_mm.tile([128, 512], F32, tag="mm512")
                        for e in range(NE):
                            nc.tensor.matmul(ps[:], wq[e][:, 128 * hp:128 * (hp + 1)],
                                             xt[e][:, sl],
                                             start=(e == 0), stop=(e == NE - 1))
                        qraw = wp.tile([128, 512], BF, tag="qraw")
                        nc.any.tensor_copy(qraw[:], ps[:])
                        qsw = wp.tile([128, 512], BF, tag="qsw")
                        for u in range(2):
                            nc.sync.dma_start(qsw[64 * u:64 * u + 32, :],
                                              qraw[64 * u + 32:64 * u + 64, :])
                            nc.sync.dma_start(qsw[64 * u + 32:64 * u + 64, :],
                                              qraw[64 * u:64 * u + 32, :])
                        t1 = wp.tile([128, 512], BF, tag="qt1")
                        nc.vector.tensor_mul(t1[:], qraw[:], cs[b][hp][:, 512 * tq2:512 * (tq2 + 1)])
                        t2 = wp.tile([128, 512], BF, tag="qt2")
                        nc.vector.tensor_mul(t2[:], qsw[:], sn[b][hp][:, 512 * tq2:512 * (tq2 + 1)])
                        nc.vector.tensor_add(qt[:, 512 * tq2:512 * (tq2 + 1)], t1[:], t2[:])
                # v projection (natural layout) -> vals[b] chunks 8..15
                for tc8 in range(S // 128):
                    ps = ps_mm.tile([128, 512], F32, tag="mm512")
                    for e in range(NE):
                        nc.tensor.matmul(ps[:, 0:64],
                                         xt[e][:, tok0 + 128 * tc8:tok0 + 128 * (tc8 + 1)],
                                         wv[e][:],
                                         start=(e == 0), stop=(e == NE - 1))
                    kk = P // 128 + tc8
                    nc.any.tensor_copy(vals[b][:, 65 * kk:65 * kk + 64], ps[:, 0:64])

            # ---- attention + output projection ----
            for b in range(B):
                for tqb in range(NTQB):
                    at_tiles = []
                    for hp in range(2):
                        at = ap.tile([128, 512], BF, tag="attn")
                        at_tiles.append(at)
                        for he in range(2):
                            qsl = qp[b][hp][64 * he:64 * (he + 1),
                                            512 * tqb:512 * (tqb + 1)]
                            pav = ps_av.tile([128, 512], F32, tag="av")
                            first = True
                            glist = _groups(cls, tqb)
                            for gi, chunks in enumerate(glist):
                                psc = ps_sc.tile([128, 2048], F32, tag="scores")
                                for j, c in enumerate(chunks):
                                    nc.tensor.matmul(
                                        psc[:, 512 * j:512 * (j + 1)],
                                        keys[b][64 * he:64 * (he + 1), 128 * c:128 * (c + 1)],
                                        qsl, start=True, stop=True)
                                prb = prp.tile([128, 2048], BF, tag="probs")
                                nc.scalar.activation(prb[:], psc[:], Exp, scale=0.125)
                                for j, c in enumerate(chunks):
                                    k_ = cls[(tqb, c)]
                                    if k_ == "mixed":
                                        nc.vector.tensor_mul(
                                            prb[:, 512 * j:512 * (j + 1)],
                                            prb[:, 512 * j:512 * (j + 1)],
                                            mt[midx[(tqb, c)]][:])
                                    elif k_ == "zero":
                                        nc.vector.memset(prb[:, 512 * j:512 * (j + 1)], 0.0)
                                last_g = gi == len(glist) - 1
                                for j, c in enumerate(chunks):
                                    nc.tensor.matmul(
                                        pav[0:65, :],
                                        vals[b][:, 65 * c:65 * (c + 1)],
                                        prb[:, 512 * j:512 * (j + 1)],
                                        start=first, stop=(last_g and j == 3))
                                    first = False
                            rcp = wp.tile([1, 512], F32, tag="rcp")
                            nc.vector.reciprocal(rcp[:], pav[64:65, :])
                            dnb = wp.tile([64, 512], F32, tag="dnb")
                            nc.gpsimd.partition_broadcast(dnb[:], rcp[:])
                            if he == 0:
                                nc.vector.tensor_mul(at[0:64, :], pav[0:64, :], dnb[:])
                            else:
                                tmo = wp.tile([64, 512], BF, tag="tmo")
                                nc.vector.tensor_mul(tmo[:], pav[0:64, :], dnb[:])
                                nc.sync.dma_start(at[64:128, :], tmo[:])
                    # Wo for this (b, tqb)
                    for t4 in range(4):
                        for eb in range(4):
                            po = ps_mm.tile([128, 512], F32, tag="mm512")
                            nc.tensor.matmul(po[:], at_tiles[0][:, 128 * t4:128 * (t4 + 1)],
                                             wo[0][:, 512 * eb:512 * (eb + 1)],
                                             start=True, stop=False)
                            nc.tensor.matmul(po[:], at_tiles[1][:, 128 * t4:128 * (t4 + 1)],
                                             wo[1][:, 512 * eb:512 * (eb + 1)],
                                             start=False, stop=True)
                            ost = op_.tile([128, 512], F32, tag="ost")
                            nc.any.tensor_copy(ost[:], po[:])
                            r0 = b * S + 512 * tqb + 128 * t4
                            nc.sync.dma_start(out_part[r0:r0 + 128, 512 * eb:512 * (eb + 1)],
                                              ost[:])
    nc.compile()
    return nc


def _prep(x, cos, sin, mask, cache_k, cache_v, Wq, Wk, Wv, Wo):
    """Host-side sharding/layout prep. Returns (cls, mixed_list, in_maps)."""
    xf = np.asarray(x, np.float32).reshape(T, E)
    xTn = np.ascontiguousarray(xf.T).astype(nbf)
    Mm = np.exp(np.asarray(mask, np.float32)[0, 0])          # [S, CTX]
    MT = np.ascontiguousarray(Mm.T)                          # [CTX, S]
    cls = _classify(MT)
    mixed_list = sorted(tc for tc, v in cls.items() if v == "mixed")

    sign = np.concatenate([-np.ones(HD // 2, np.float32), np.ones(HD // 2, np.float32)])
    cosn = np.asarray(cos, np.float32)
    sinn = np.asarray(sin, np.float32)
    cosRn = np.stack([np.tile(cosn[b].T, (G, 1)) for b in range(B)]).astype(nbf)
    ssinRn = np.stack([np.tile(sign[:, None] * sinn[b].T, (G, 1)) for b in range(B)]).astype(nbf)

    maskM_np = None
    if mixed_list:
        maskM_np = np.stack([
            MT[128 * c:128 * (c + 1), 512 * tqb:512 * (tqb + 1)]
            for (tqb, c) in mixed_list]).astype(nbf)

    Wqn = np.asarray(Wq, np.float32)
    Wkn = np.asarray(Wk, np.float32)
    Wvn = np.asarray(Wv, np.float32)
    Won = np.asarray(Wo, np.float32)
    ckn = np.asarray(cache_k, np.float32)
    cvn = np.asarray(cache_v, np.float32)

    in_maps = []
    for c in range(N_CORES):
        m = {
            "xT": xTn,
            "wqT": np.ascontiguousarray(Wqn[c * OC:(c + 1) * OC].T).astype(nbf),
            "wkT": np.ascontiguousarray(Wkn[c * HD:(c + 1) * HD].T).astype(nbf),
            "wvT": np.ascontiguousarray(Wvn[c * HD:(c + 1) * HD].T).astype(nbf),
            "woT": np.ascontiguousarray(Won[:, c * OC:(c + 1) * OC].T).astype(nbf),
            "cosR": cosRn,
            "ssinR": ssinRn,
            "cacheTk": np.ascontiguousarray(ckn[:, c, :P].transpose(0, 2, 1)).astype(nbf),
            "cacheV": np.ascontiguousarray(cvn[:, c, :P]).astype(nbf),
        }
        if maskM_np is not None:
            m["maskM"] = maskM_np
        in_maps.append(m)
    return cls, mixed_list, in_maps


def kernel(x, cos, sin, mask, cache_k, cache_v, Wq, Wk, Wv, Wo, start_pos):
    assert int(start_pos) == P, f"kernel hardcodes start_pos={P}, got {start_pos}"
    cls, mixed_list, in_maps = _prep(x, cos, sin, mask, cache_k, cache_v,
                                     Wq, Wk, Wv, Wo)
    key = tuple(sorted(cls.items()))
    if key not in _built:
        _built[key] = _build(cls, mixed_list)
    nc = _built[key]
    res = run_bass_kernel_spmd(nc, in_maps, core_ids=list(range(N_CORES)))
    acc = res.results[0]["out_part"].astype(np.float32).copy()
    for c in range(1, N_CORES):
        acc += res.results[c]["out_part"]
    return acc.reshape(B, S, E)


# revision 10
# speedup vs baseline: 1.3508x; 1.3508x over previous
"""Llama GQA attention (B=2,S=1024,P=1024,E=2048,H=32,KV=8,HD=64) on 8 TRN2 cores.

Sharding: tensor-parallel on the KV-group axis — core c owns KV group c and its
4 query heads. x / cos / sin / mask replicated; Wq/Wk/Wv row-sharded; Wo
column-sharded (partial outputs summed on host); cache sharded on the KV axis.
"""
import os
import sys

for _p in ("/opt/trn_rl_repo",):
    if os.path.isdir(_p) and _p not in sys.path:
        sys.path.insert(0, _p)

import numpy as np
import ml_dtypes

import concourse.bass as bass
import concourse.tile as tile
from concourse import bacc, mybir
from concourse.bass_utils import run_bass_kernel_spmd

B, S, P, E, H, KV, HD = 2, 1024, 1024, 2048, 32, 8, 64
CTX = P + S            # 2048
G = H // KV            # 4 heads per core
T = B * S              # 2048 flattened tokens
N_CORES = 8
OC = G * HD            # 256 output cols per core (q / attn)
BF = mybir.dt.bfloat16
F32 = mybir.dt.float32
nbf = ml_dtypes.bfloat16

NCH = CTX // 128       # 16 key chunks of 128
NTQB = S // 512        # 2 query blocks of 512
NE = E // 128          # 16 embed chunks

_built = {}            # classification key -> compiled Bass module


def _classify(MT):
    """MT = exp(mask).T, shape [CTX, S]. Per (tqb, chunk): 'ones'|'zero'|'mixed'."""
    cls = {}
    for tqb in range(NTQB):
        for c in range(NCH):
            sub = MT[128 * c:128 * (c + 1), 512 * tqb:512 * (tqb + 1)]
            if np.all(sub == 1.0):
                cls[(tqb, c)] = "ones"
            elif np.all(sub == 0.0):
                cls[(tqb, c)] = "zero"
            else:
                cls[(tqb, c)] = "mixed"
    return cls


def _groups(cls, tqb, grp):
    """Key-chunk groups of `grp` for one tq block; a group is skipped only if
    all its chunks are fully masked ('zero')."""
    out = []
    for g in range(NCH // grp):
        chunks = list(range(grp * g, grp * (g + 1)))
        if all(cls[(tqb, c)] == "zero" for c in chunks):
            continue
        out.append(chunks)
    return out


DEFAULT_OPTS = dict(
    grp=2,          # key chunks per score group (psum banks per scores buf)
    sc_bufs=2,      # scores psum bufs
    nonorm=False,   # skip softmax normalization (ablation only — wrong result)
    phase=3,        # ablation: 1=loads only, 2=+projections, 3=full
    direct_odd=True,   # write odd head's normalize output straight to at[64:128]
    no_exp=False, no_mask=False, no_av=False, no_scoremm=False, no_wo=False,
    probs_bufs=3, ilv=True, dma_spread=True,
    evac_engine="any",  # engine for psum->sbuf copies: any|vector|scalar
)


def _build(cls, mixed_list, opts=None):
    o = dict(DEFAULT_OPTS)
    if opts:
        o.update(opts)
    Exp = mybir.ActivationFunctionType.Exp
    midx = {tc: j for j, tc in enumerate(mixed_list)}
    nc = bacc.Bacc(None, target_bir_lowering=False, debug=False)
    _evac = {"any": lambda: nc.any, "vector": lambda: nc.vector,
             "scalar": lambda: nc.scalar}[o["evac_engine"]]
    def evac(out, in_):
        if o["evac_engine"] == "scalar":
            nc.scalar.copy(out, in_)
        else:
            _evac().tensor_copy(out, in_)

    xT = nc.dram_tensor("xT", [E, T], BF, kind="ExternalInput")
    wqT = nc.dram_tensor("wqT", [E, OC], BF, kind="ExternalInput")
    wkT = nc.dram_tensor("wkT", [E, HD], BF, kind="ExternalInput")
    wvT = nc.dram_tensor("wvT", [E, HD], BF, kind="ExternalInput")
    woT = nc.dram_tensor("woT", [OC, E], BF, kind="ExternalInput")
    cosR = nc.dram_tensor("cosR", [B, OC, S], BF, kind="ExternalInput")
    ssinR = nc.dram_tensor("ssinR", [B, OC, S], BF, kind="ExternalInput")
    cacheTk = nc.dram_tensor("cacheTk", [B, HD, P], BF, kind="ExternalInput")
    cacheV = nc.dram_tensor("cacheV", [B, P, HD], BF, kind="ExternalInput")
    if mixed_list:
        maskM = nc.dram_tensor("maskM", [len(mixed_list), 128, 512], BF,
                               kind="ExternalInput")
    out_part = nc.dram_tensor("out_part", [T, E], F32, kind="ExternalOutput")

    with tile.TileContext(nc) as tc:
        with (
            tc.tile_pool(name="persist", bufs=1) as pp,
            tc.tile_pool(name="work", bufs=3) as wp,
            tc.tile_pool(name="probs", bufs=o["probs_bufs"]) as prp,
            tc.tile_pool(name="attn", bufs=4) as ap,
            tc.tile_pool(name="ostage", bufs=4) as op_,
            tc.tile_pool(name="ps_sc", bufs=o["sc_bufs"], space="PSUM") as ps_sc,
            tc.tile_pool(name="ps_av", bufs=2, space="PSUM") as ps_av,
            tc.tile_pool(name="ps_mm", bufs=2, space="PSUM") as ps_mm,
        ):
            # ---- persistent loads ----
            if o["dma_spread"]:
                _dmaeng = [nc.sync, nc.scalar, nc.gpsimd]
            else:
                _dmaeng = [nc.sync]
            _dmac = [0]
            def ldma(out, in_):
                e_ = _dmaeng[_dmac[0] % len(_dmaeng)]
                _dmac[0] += 1
                e_.dma_start(out, in_)
            xt = []
            for i in range(NE):
                t_ = pp.tile([128, T], BF, tag=f"xt{i}")
                ldma(t_[:], xT[128 * i:128 * (i + 1), :])
                xt.append(t_)
            wq = []
            for i in range(NE):
                t_ = pp.tile([128, OC], BF, tag=f"wq{i}")
                ldma(t_[:], wqT[128 * i:128 * (i + 1), :])
                wq.append(t_)
            wk, wv = [], []
            for i in range(NE):
                t_ = pp.tile([128, HD], BF, tag=f"wk{i}")
                ldma(t_[:], wkT[128 * i:128 * (i + 1), :])
                wk.append(t_)
                t_ = pp.tile([128, HD], BF, tag=f"wv{i}")
                ldma(t_[:], wvT[128 * i:128 * (i + 1), :])
                wv.append(t_)
            wo = []
            for i in range(2):
                t_ = pp.tile([128, E], BF, tag=f"wo{i}")
                ldma(t_[:], woT[128 * i:128 * (i + 1), :])
                wo.append(t_)
            cs, sn = [], []
            for b in range(B):
                cb, sb_ = [], []
                for hp in range(2):
                    t_ = pp.tile([128, S], BF, tag=f"cos{b}{hp}")
                    ldma(t_[:], cosR[b, 128 * hp:128 * (hp + 1), :])
                    cb.append(t_)
                    t_ = pp.tile([128, S], BF, tag=f"sin{b}{hp}")
                    ldma(t_[:], ssinR[b, 128 * hp:128 * (hp + 1), :])
                    sb_.append(t_)
                cs.append(cb)
                sn.append(sb_)
            keys, vals = [], []
            for b in range(B):
                kt = pp.tile([128, CTX], BF, tag=f"keys{b}")
                ldma(kt[0:64, 0:P], cacheTk[b])
                ldma(kt[64:128, 0:P], cacheTk[b])
                keys.append(kt)
                vt = pp.tile([128, NCH * 65], BF, tag=f"vals{b}")
                for k in range(P // 128):
                    ldma(vt[:, 65 * k:65 * k + 64],
                                      cacheV[b, 128 * k:128 * (k + 1), :])
                for k in range(NCH):
                    nc.vector.memset(vt[:, 65 * k + 64:65 * k + 65], 1.0)
                vals.append(vt)
            mt = []
            for j in range(len(mixed_list)):
                t_ = pp.tile([128, 512], BF, tag=f"mask{j}")
                ldma(t_[:], maskM[j])
                mt.append(t_)

            # ---- projections + RoPE ----
            qp = [[None, None] for _ in range(B)]
            for b in range(B if o["phase"] >= 2 else 0):
                tok0 = b * S
                # k projection (transposed) + rope -> keys[b][:, P:]
                for tq2 in range(2):
                    sl = slice(tok0 + 512 * tq2, tok0 + 512 * (tq2 + 1))
                    ps = ps_mm.tile([128, 512], F32, tag="mm512")
                    for e in range(NE):
                        nc.tensor.matmul(ps[0:64, :], wk[e][:, 0:64], xt[e][:, sl],
                                         start=(e == 0), stop=(e == NE - 1))
                    kraw = wp.tile([64, 512], BF, tag="kraw")
                    evac(kraw[:], ps[0:64, :])
                    ksw = wp.tile([64, 512], BF, tag="ksw")
                    nc.sync.dma_start(ksw[0:32, :], kraw[32:64, :])
                    nc.sync.dma_start(ksw[32:64, :], kraw[0:32, :])
                    t1 = wp.tile([64, 512], BF, tag="kt1")
                    nc.vector.tensor_mul(t1[:], kraw[:], cs[b][0][0:64, 512 * tq2:512 * (tq2 + 1)])
                    t2 = wp.tile([64, 512], BF, tag="kt2")
                    nc.vector.tensor_mul(t2[:], ksw[:], sn[b][0][0:64, 512 * tq2:512 * (tq2 + 1)])
                    ksl = slice(P + 512 * tq2, P + 512 * (tq2 + 1))
                    nc.vector.tensor_add(keys[b][0:64, ksl], t1[:], t2[:])
                    nc.sync.dma_start(keys[b][64:128, ksl], keys[b][0:64, ksl])
                # q projection (transposed, head-pair packed) + rope
                for hp in range(2):
                    qt = pp.tile([128, S], BF, tag=f"qp{b}{hp}")
                    qp[b][hp] = qt
                    for tq2 in range(2):
                        sl = slice(tok0 + 512 * tq2, tok0 + 512 * (tq2 + 1))
                        ps = ps_mm.tile([128, 512], F32, tag="mm512")
                        for e in range(NE):
                            nc.tensor.matmul(ps[:], wq[e][:, 128 * hp:128 * (hp + 1)],
                                             xt[e][:, sl],
                                             start=(e == 0), stop=(e == NE - 1))
                        qraw = wp.tile([128, 512], BF, tag="qraw")
                        evac(qraw[:], ps[:])
                        qsw = wp.tile([128, 512], BF, tag="qsw")
                        for u in range(2):
                            nc.sync.dma_start(qsw[64 * u:64 * u + 32, :],
                                              qraw[64 * u + 32:64 * u + 64, :])
                            nc.sync.dma_start(qsw[64 * u + 32:64 * u + 64, :],
                                              qraw[64 * u:64 * u + 32, :])
                        t1 = wp.tile([128, 512], BF, tag="qt1")
                        nc.vector.tensor_mul(t1[:], qraw[:], cs[b][hp][:, 512 * tq2:512 * (tq2 + 1)])
                        t2 = wp.tile([128, 512], BF, tag="qt2")
                        nc.vector.tensor_mul(t2[:], qsw[:], sn[b][hp][:, 512 * tq2:512 * (tq2 + 1)])
                        nc.vector.tensor_add(qt[:, 512 * tq2:512 * (tq2 + 1)], t1[:], t2[:])
                # v projection (natural layout) -> vals[b] chunks 8..15
                for tc8 in range(S // 128):
                    ps = ps_mm.tile([128, 512], F32, tag="mm512")
                    for e in range(NE):
                        nc.tensor.matmul(ps[:, 0:64],
                                         xt[e][:, tok0 + 128 * tc8:tok0 + 128 * (tc8 + 1)],
                                         wv[e][:],
                                         start=(e == 0), stop=(e == NE - 1))
                    kk = P // 128 + tc8
                    evac(vals[b][:, 65 * kk:65 * kk + 64], ps[:, 0:64])

            # ---- attention + output projection ----
            for b in range(B if o["phase"] >= 3 else 0):
                for tqb in range(NTQB):
                    at_tiles = []
                    glist = _groups(cls, tqb, o["grp"])
                    for hp in range(2):
                        at = ap.tile([128, 512], BF, tag="attn")
                        at_tiles.append(at)
                        if o["ilv"]:
                            pavs = [ps_av.tile([128, 512], F32, tag="av",
                                               name=f"pav{b}{tqb}{hp}{i_}")
                                    for i_ in range(2)]
                            first = [True, True]
                            for gi, chunks in enumerate(glist):
                                last_g = gi == len(glist) - 1
                                for he in range(2):
                                    qsl = qp[b][hp][64 * he:64 * (he + 1),
                                                    512 * tqb:512 * (tqb + 1)]
                                    psc = ps_sc.tile([128, 512 * o["grp"]], F32,
                                                     tag="scores")
                                    for j, c in enumerate(chunks):
                                        nc.tensor.matmul(
                                            psc[:, 512 * j:512 * (j + 1)],
                                            keys[b][64 * he:64 * (he + 1),
                                                    128 * c:128 * (c + 1)],
                                            qsl, start=True, stop=True)
                                    prb = prp.tile([128, 512 * o["grp"]], BF,
                                                   tag="probs")
                                    nc.scalar.activation(prb[:], psc[:], Exp,
                                                         scale=0.125)
                                    for j, c in enumerate(chunks):
                                        k_ = cls[(tqb, c)]
                                        if k_ == "mixed":
                                            nc.vector.tensor_mul(
                                                prb[:, 512 * j:512 * (j + 1)],
                                                prb[:, 512 * j:512 * (j + 1)],
                                                mt[midx[(tqb, c)]][:])
                                        elif k_ == "zero":
                                            nc.vector.memset(
                                                prb[:, 512 * j:512 * (j + 1)], 0.0)
                                    for j, c in enumerate(chunks):
                                        nc.tensor.matmul(
                                            pavs[he][0:65, :],
                                            vals[b][:, 65 * c:65 * (c + 1)],
                                            prb[:, 512 * j:512 * (j + 1)],
                                            start=first[he],
                                            stop=(last_g and j == len(chunks) - 1))
                                        first[he] = False
                            for he in range(2):
                                rcp = wp.tile([1, 512], F32, tag="rcp")
                                nc.vector.reciprocal(rcp[:], pavs[he][64:65, :])
                                dnb = wp.tile([64, 512], F32, tag="dnb")
                                nc.gpsimd.partition_broadcast(dnb[:], rcp[:])
                                nc.vector.tensor_mul(at[64 * he:64 * (he + 1), :],
                                                     pavs[he][0:64, :], dnb[:])
                            continue
                        for he in range(2):
                            qsl = qp[b][hp][64 * he:64 * (he + 1),
                                            512 * tqb:512 * (tqb + 1)]
                            pav = ps_av.tile([128, 512], F32, tag="av")
                            first = True
                            for gi, chunks in enumerate(glist):
                                psc = ps_sc.tile([128, 512 * o["grp"]], F32, tag="scores")
                                for j, c in enumerate(chunks):
                                    if o["no_scoremm"]:
                                        break
                                    nc.tensor.matmul(
                                        psc[:, 512 * j:512 * (j + 1)],
                                        keys[b][64 * he:64 * (he + 1), 128 * c:128 * (c + 1)],
                                        qsl, start=True, stop=True)
                                prb = prp.tile([128, 512 * o["grp"]], BF, tag="probs")
                                if o["no_exp"]:
                                    nc.vector.tensor_copy(prb[:], psc[:])
                                else:
                                    nc.scalar.activation(prb[:], psc[:], Exp, scale=0.125)
                                for j, c in enumerate(chunks):
                                    k_ = cls[(tqb, c)]
                                    if k_ == "mixed" and not o["no_mask"]:
                                        nc.vector.tensor_mul(
                                            prb[:, 512 * j:512 * (j + 1)],
                                            prb[:, 512 * j:512 * (j + 1)],
                                            mt[midx[(tqb, c)]][:])
                                    elif k_ == "zero":
                                        nc.vector.memset(prb[:, 512 * j:512 * (j + 1)], 0.0)
                                last_g = gi == len(glist) - 1
                                for j, c in enumerate(chunks):
                                    if o["no_av"]:
                                        break
                                    nc.tensor.matmul(
                                        pav[0:65, :],
                                        vals[b][:, 65 * c:65 * (c + 1)],
                                        prb[:, 512 * j:512 * (j + 1)],
                                        start=first,
                                        stop=(last_g and j == len(chunks) - 1))
                                    first = False
                            if o["no_av"]:
                                continue
                            if o["nonorm"]:
                                evac(at[64 * he:64 * (he + 1), :], pav[0:64, :])
                                continue
                            rcp = wp.tile([1, 512], F32, tag="rcp")
                            nc.vector.reciprocal(rcp[:], pav[64:65, :])
                            dnb = wp.tile([64, 512], F32, tag="dnb")
                            nc.gpsimd.partition_broadcast(dnb[:], rcp[:])
                            if he == 0 or o["direct_odd"]:
                                nc.vector.tensor_mul(at[64 * he:64 * (he + 1), :],
                                                     pav[0:64, :], dnb[:])
                            else:
                                tmo = wp.tile([64, 512], BF, tag="tmo")
                                nc.vector.tensor_mul(tmo[:], pav[0:64, :], dnb[:])
                                nc.sync.dma_start(at[64:128, :], tmo[:])
                    # Wo for this (b, tqb)
                    for t4 in range(0 if o["no_wo"] else 4):
                        for eb in range(4):
                            po = ps_mm.tile([128, 512], F32, tag="mm512")
                            nc.tensor.matmul(po[:], at_tiles[0][:, 128 * t4:128 * (t4 + 1)],
                                             wo[0][:, 512 * eb:512 * (eb + 1)],
                                             start=True, stop=False)
                            nc.tensor.matmul(po[:], at_tiles[1][:, 128 * t4:128 * (t4 + 1)],
                                             wo[1][:, 512 * eb:512 * (eb + 1)],
                                             start=False, stop=True)
                            ost = op_.tile([128, 512], F32, tag="ost")
                            evac(ost[:], po[:])
                            r0 = b * S + 512 * tqb + 128 * t4
                            nc.sync.dma_start(out_part[r0:r0 + 128, 512 * eb:512 * (eb + 1)],
                                              ost[:])
    nc.compile()
    return nc


def _prep(x, cos, sin, mask, cache_k, cache_v, Wq, Wk, Wv, Wo):
    """Host-side sharding/layout prep. Returns (cls, mixed_list, in_maps)."""
    xf = np.asarray(x, np.float32).reshape(T, E)
    xTn = np.ascontiguousarray(xf.T).astype(nbf)
    Mm = np.exp(np.asarray(mask, np.float32)[0, 0])          # [S, CTX]
    MT = np.ascontiguousarray(Mm.T)                          # [CTX, S]
    cls = _classify(MT)
    mixed_list = sorted(tc for tc, v in cls.items() if v == "mixed")

    sign = np.concatenate([-np.ones(HD // 2, np.float32), np.ones(HD // 2, np.float32)])
    cosn = np.asarray(cos, np.float32)
    sinn = np.asarray(sin, np.float32)
    cosRn = np.stack([np.tile(cosn[b].T, (G, 1)) for b in range(B)]).astype(nbf)
    ssinRn = np.stack([np.tile(sign[:, None] * sinn[b].T, (G, 1)) for b in range(B)]).astype(nbf)

    maskM_np = None
    if mixed_list:
        maskM_np = np.stack([
            MT[128 * c:128 * (c + 1), 512 * tqb:512 * (tqb + 1)]
            for (tqb, c) in mixed_list]).astype(nbf)

    Wqn = np.asarray(Wq, np.float32)
    Wkn = np.asarray(Wk, np.float32)
    Wvn = np.asarray(Wv, np.float32)
    Won = np.asarray(Wo, np.float32)
    ckn = np.asarray(cache_k, np.float32)
    cvn = np.asarray(cache_v, np.float32)

    in_maps = []
    for c in range(N_CORES):
        m = {
            "xT": xTn,
            "wqT": np.ascontiguousarray(Wqn[c * OC:(c + 1) * OC].T).astype(nbf),
            "wkT": np.ascontiguousarray(Wkn[c * HD:(c + 1) * HD].T).astype(nbf),
            "wvT": np.ascontiguousarray(Wvn[c * HD:(c + 1) * HD].T).astype(nbf),
            "woT": np.ascontiguousarray(Won[:, c * OC:(c + 1) * OC].T).astype(nbf),
            "cosR": cosRn,
            "ssinR": ssinRn,
            "cacheTk": np.ascontiguousarray(ckn[:, c, :P].transpose(0, 2, 1)).astype(nbf),
            "cacheV": np.ascontiguousarray(cvn[:, c, :P]).astype(nbf),
        }
        if maskM_np is not None:
            m["maskM"] = maskM_np
        in_maps.append(m)
    return cls, mixed_list, in_maps


def kernel(x, cos, sin, mask, cache_k, cache_v, Wq, Wk, Wv, Wo, start_pos):
    assert int(start_pos) == P, f"kernel hardcodes start_pos={P}, got {start_pos}"
    cls, mixed_list, in_maps = _prep(x, cos, sin, mask, cache_k, cache_v,
                                     Wq, Wk, Wv, Wo)
    key = tuple(sorted(cls.items()))
    if key not in _built:
        _built[key] = _build(cls, mixed_list)
    nc = _built[key]
    res = run_bass_kernel_spmd(nc, in_maps, core_ids=list(range(N_CORES)))
    acc = res.results[0]["out_part"].astype(np.float32).copy()
    for c in range(1, N_CORES):
        acc += res.results[c]["out_part"]
    return acc.reshape(B, S, E)
